# revision 8
# baseline (speedup 1.0000x reference)
"""Show-Attend-Tell decoder on 8 TRN2 NeuronCores.

Sharding: data-parallel recurrence (8 batches/core, attention+LSTM fully
local), deep-output/vocab projection batched at the END: one AllGather of
per-step proj inputs, then each core computes its V/8 = 1250 vocab shard
for all 64 batches with full-M matmuls.

Shapes (hardcoded): V=10000 E=D=A=ENC=512 B=64 L=196 T=20.
"""
import numpy as np
import ml_dtypes
from contextlib import ExitStack

V, E, D, A, ENC = 10000, 512, 512, 512, 512
B, L, T = 64, 196, 20
NCORES = 8
BL = B // NCORES          # 8 local batches
LP = 256                  # padded per-batch L
NT = T - 1                # 19 steps
ROWS = NT * BL            # 152 (t-major rows t*8+b)
VS = V // NCORES          # 1250 vocab shard

_BF = ml_dtypes.bfloat16
_CACHE = {}


def _build():
    import concourse.mybir as mybir
    import concourse.tile as tile
    from concourse import bacc

    f32 = mybir.dt.float32
    bf16 = mybir.dt.bfloat16
    i32 = mybir.dt.int32

    nc = bacc.Bacc("TRN2", target_bir_lowering=False, debug=False,
                   num_devices=NCORES)

    io = {}
    def din(name, shape, dt):
        io[name] = nc.dram_tensor(name, shape, dt, kind="ExternalInput")
    din("fstk", [BL * LP, ENC], bf16)
    din("capi", [ROWS, 1], i32)
    din("emb", [V, E], f32)
    din("wenc", [ENC, A], bf16)
    din("baed", [128, 4], f32)
    din("wad", [D, A], bf16)
    din("wfull", [128, 4], bf16)
    din("wbeta", [D, ENC], bf16)
    din("bbeta", [1, ENC], bf16)
    din("wzh", [ENC + D, 4 * D], bf16)
    din("wihx", [E, 4 * D], bf16)
    din("gbias", [1, 4 * D], bf16)
    din("winh", [ENC, D], f32)
    din("binh", [1, D], f32)
    din("winc", [ENC, D], f32)
    din("binc", [1, D], f32)
    din("wlh", [D, E], bf16)
    din("wlz", [ENC, E], bf16)
    din("blhz", [1, E], bf16)
    din("wlo", [E, VS], bf16)
    din("blo", [1, VS], f32)
    io["logits_s"] = nc.dram_tensor("logits_s", [B, NT, VS], f32, kind="ExternalOutput")
    io["dbg_proj"] = nc.dram_tensor("dbg_proj", [ROWS, E], f32, kind="ExternalOutput")
    io["dbg_pag"] = nc.dram_tensor("dbg_pag", [NCORES * ROWS, E], f32, kind="ExternalOutput")
    io["alphas_s"] = nc.dram_tensor("alphas_s", [BL, NT, L], f32, kind="ExternalOutput")

    with tile.TileContext(nc) as tc:
        _body(nc, tc, io)
    nc.compile()
    return nc


def _body(nc, tc, io):
    import concourse.bass as bass
    import concourse.mybir as mybir

    f32 = mybir.dt.float32
    bf16 = mybir.dt.bfloat16
    i32 = mybir.dt.int32
    AF = mybir.ActivationFunctionType
    OP = mybir.AluOpType
    ds, ts = bass.ds, bass.ts
    X = mybir.AxisListType.X

    with ExitStack() as stk:
        const = stk.enter_context(tc.tile_pool(name="const", bufs=1))
        wpool = stk.enter_context(tc.tile_pool(name="wpool", bufs=1))
        state = stk.enter_context(tc.tile_pool(name="state", bufs=1))
        dram = stk.enter_context(tc.tile_pool(name="dram", bufs=1, space="DRAM"))

        from concourse.masks import make_identity
        idf = const.tile([128, 128], f32)
        make_identity(nc, idf)
        idb = const.tile([128, 128], bf16)
        make_identity(nc, idb)
        onesb = const.tile([1, 128], bf16)
        nc.gpsimd.memset(onesb[:, :], 1.0)
        onesf = const.tile([1, 8], f32)
        nc.gpsimd.memset(onesf[:, :], 1.0)

        def dma_cpe(dst, name):
            nc.sync.dma_start(dst[:, :, :], io[name].ap().rearrange("(c p) e -> p c e", p=128))

        fstk = wpool.tile([128, 16, ENC], bf16); dma_cpe(fstk, "fstk")
        wad = wpool.tile([128, 4, A], bf16); dma_cpe(wad, "wad")
        wbeta = wpool.tile([128, 4, ENC], bf16); dma_cpe(wbeta, "wbeta")
        wzh = wpool.tile([128, 8, 4 * D], bf16); dma_cpe(wzh, "wzh")
        wlh = wpool.tile([128, 4, E], bf16); dma_cpe(wlh, "wlh")
        wlz = wpool.tile([128, 4, E], bf16); dma_cpe(wlz, "wlz")
        wlo = wpool.tile([128, 4, VS], bf16); dma_cpe(wlo, "wlo")
        wfull = wpool.tile([128, 4], bf16)
        nc.sync.dma_start(wfull[:, :], io["wfull"].ap())
        baed = wpool.tile([128, 4], f32)
        nc.sync.dma_start(baed[:, :], io["baed"].ap())
        bbeta = wpool.tile([1, ENC], bf16)
        nc.sync.dma_start(bbeta[:, :], io["bbeta"].ap())
        blhz = wpool.tile([1, E], bf16)
        nc.sync.dma_start(blhz[:, :], io["blhz"].ap())
        bloBC = wpool.tile([128, VS], f32)
        blo_row = wpool.tile([1, VS], f32)
        nc.sync.dma_start(blo_row[:, :], io["blo"].ap())
        nc.gpsimd.partition_broadcast(bloBC[:, :], blo_row[:, :])

        fpT = state.tile([128, 4, BL * LP], bf16)
        x0 = state.tile([128, ENC], f32)
        x1 = state.tile([24, ENC], f32)
        xT = state.tile([128, 4, ROWS], bf16)
        hT_all = state.tile([128, 4, NT, BL], bf16)
        zT_all = state.tile([128, 4, NT, BL], bf16)
        hT0 = state.tile([128, 4, BL], bf16)
        c_s = state.tile([BL, D], f32)
        al0 = state.tile([128, NT, L], bf16)
        al1 = state.tile([128, NT, L], bf16)
        bd = state.tile([128, 16, BL], bf16)
        nc.gpsimd.memset(bd[:, :, :], 0.0)

        gx_dram = dram.tile([ROWS, 4 * D], f32)

        # ---------------- preamble ----------------
        with tc.tile_pool(name="pre", bufs=1) as pre, \
             tc.tile_pool(name="prep", bufs=2, space="PSUM") as prep:
            idx0 = pre.tile([128, 1], i32)
            idx1 = pre.tile([24, 1], i32)
            nc.sync.dma_start(idx0[:, :], io["capi"].ap()[0:128, :])
            nc.sync.dma_start(idx1[:, :], io["capi"].ap()[128:ROWS, :])
            nc.gpsimd.indirect_dma_start(
                out=x0[:, :], out_offset=None, in_=io["emb"].ap(),
                in_offset=bass.IndirectOffsetOnAxis(ap=idx0[:, :1], axis=0))
            nc.gpsimd.indirect_dma_start(
                out=x1[:, :], out_offset=None, in_=io["emb"].ap(),
                in_offset=bass.IndirectOffsetOnAxis(ap=idx1[:, :1], axis=0))
            for rc, (xt, nrow) in enumerate([(x0, 128), (x1, 24)]):
                for ec in range(4):
                    tp = prep.tile([128, 128], f32, tag="tp", bufs=2)
                    nc.tensor.transpose(tp[:, :nrow], xt[:nrow, ts(ec, 128)],
                                        idf[:nrow, :nrow])
                    nc.vector.tensor_copy(xT[:, ec, ds(rc * 128, nrow)], tp[:, :nrow])

            featT = pre.tile([128, 4, BL * LP], bf16)
            for c in range(16):
                for ec in range(4):
                    tp = prep.tile([128, 128], bf16, tag="tpb", bufs=2)
                    nc.tensor.transpose(tp[:, :], fstk[:, c, ts(ec, 128)], idb[:, :])
                    nc.vector.tensor_copy(featT[:, ec, ts(c, 128)], tp[:, :])

            wenc = pre.tile([128, 4, A], bf16)
            nc.sync.dma_start(wenc[:, :, :], io["wenc"].ap().rearrange("(c p) e -> p c e", p=128))
            for ac in range(4):
                for nb in range(4):
                    pp = prep.tile([128, 512], f32, tag="pp", bufs=2)
                    for kc in range(4):
                        nc.tensor.matmul(pp[:, :], wenc[:, kc, ts(ac, 128)],
                                         featT[:, kc, ts(nb, 512)],
                                         start=(kc == 0), stop=(kc == 3))
                    if (ac + nb) % 2:
                        nc.scalar.activation(fpT[:, ac, ts(nb, 512)], pp[:, :],
                                             AF.Identity, bias=baed[:, ac:ac + 1])
                    else:
                        nc.vector.tensor_scalar(fpT[:, ac, ts(nb, 512)], pp[:, :],
                                                baed[:, ac:ac + 1], None, op0=OP.add)

            wihx = pre.tile([128, 4, 4 * D], bf16)
            nc.sync.dma_start(wihx[:, :, :], io["wihx"].ap().rearrange("(c p) e -> p c e", p=128))
            gbias = pre.tile([1, 4 * D], bf16)
            nc.sync.dma_start(gbias[:, :], io["gbias"].ap())
            for mc, nrow in [(0, 128), (1, 24)]:
                stg = pre.tile([128, 4 * D], f32, tag="gstage")
                for nb in range(4):
                    pp = prep.tile([128, 512], f32, tag="pp", bufs=2)
                    for kc in range(4):
                        nc.tensor.matmul(pp[:nrow, :], xT[:, kc, ds(mc * 128, nrow)],
                                         wihx[:, kc, ts(nb, 512)],
                                         start=(kc == 0), stop=False)
                    nc.tensor.matmul(pp[:nrow, :], onesb[:, :nrow],
                                     gbias[:, ts(nb, 512)], start=False, stop=True)
                    nc.vector.tensor_copy(stg[:nrow, ts(nb, 512)], pp[:nrow, :])
                nc.sync.dma_start(gx_dram[ds(mc * 128, nrow), :], stg[:nrow, :])

            bdo = pre.tile([128, 16, BL], bf16)
            nc.gpsimd.memset(bdo[:, :, :], 0.0)
            for b in range(BL):
                nc.gpsimd.memset(bdo[:, 2 * b, b:b + 1], 1.0 / L)
                nc.gpsimd.memset(bdo[0:L - 128, 2 * b + 1, b:b + 1], 1.0 / L)
            mean_ps = prep.tile([BL, ENC], f32, tag="pz", bufs=2)
            for kc in range(16):
                nc.tensor.matmul(mean_ps[:, :], bdo[:, kc, :], fstk[:, kc, :],
                                 start=(kc == 0), stop=(kc == 15))
            mean_s = pre.tile([BL, ENC], f32)
            nc.vector.tensor_copy(mean_s[:, :], mean_ps[:, :])
            meanT = pre.tile([128, 4, BL], f32)
            for ec in range(4):
                tp = prep.tile([128, 128], f32, tag="tp", bufs=2)
                nc.tensor.transpose(tp[:, :BL], mean_s[:, ts(ec, 128)], idf[:BL, :BL])
                nc.vector.tensor_copy(meanT[:, ec, :], tp[:, :BL])
            winh = pre.tile([128, 4, D], f32)
            nc.sync.dma_start(winh[:, :, :], io["winh"].ap().rearrange("(c p) e -> p c e", p=128))
            winc = pre.tile([128, 4, D], f32)
            nc.sync.dma_start(winc[:, :, :], io["winc"].ap().rearrange("(c p) e -> p c e", p=128))
            binh = pre.tile([1, D], f32)
            nc.sync.dma_start(binh[:, :], io["binh"].ap())
            binc = pre.tile([1, D], f32)
            nc.sync.dma_start(binc[:, :], io["binc"].ap())
            h0b = pre.tile([BL, D], bf16)
            for w_, b_, is_h in [(winh, binh, True), (winc, binc, False)]:
                pp = prep.tile([BL, D], f32, tag="pz", bufs=2)
                for kc in range(4):
                    nc.tensor.matmul(pp[:, :], meanT[:, kc, :], w_[:, kc, :],
                                     start=(kc == 0), stop=False)
                nc.tensor.matmul(pp[:, :], onesf[:, :BL], b_[:, :],
                                 start=False, stop=True)
                if is_h:
                    nc.scalar.activation(h0b[:, :], pp[:, :], AF.Tanh)
                else:
                    nc.scalar.activation(c_s[:, :], pp[:, :], AF.Tanh)
            for ec in range(4):
                tp = prep.tile([128, 128], bf16, tag="tpb", bufs=2)
                nc.tensor.transpose(tp[:, :BL], h0b[:, ts(ec, 128)], idb[:BL, :BL])
                nc.vector.tensor_copy(hT0[:, ec, :], tp[:, :BL])

            dummy = pre.tile([1, 1], f32)
            nc.scalar.activation(dummy[:, :], dummy[:, :], AF.Exp)

        # ---------------- recurrence ----------------
        with tc.tile_pool(name="work", bufs=2) as work, \
             tc.tile_pool(name="gxp", bufs=2) as gxp, \
             tc.tile_pool(name="psMix", bufs=2, space="PSUM") as psMix, \
             tc.tile_pool(name="psG", bufs=1, space="PSUM") as psG:
            for t in range(NT):
                def hTs(kc, _t=t):
                    return hT0[:, kc, :] if _t == 0 else hT_all[:, kc, _t - 1, :]

                qT_ps = psMix.tile([128, 4 * BL], f32, tag="mix", bufs=2)
                for ac in range(4):
                    for kc in range(4):
                        nc.tensor.matmul(qT_ps[:, ds(ac * BL, BL)],
                                         wad[:, kc, ts(ac, 128)], hTs(kc),
                                         start=(kc == 0), stop=(kc == 3))
                qT = work.tile([128, 4 * BL], f32, tag="qT")
                nc.vector.tensor_copy(qT[:, :], qT_ps[:, :])

                attT = work.tile([128, 4, BL * LP], bf16, tag="attT", bufs=1)
                for b in range(BL):
                    for ac in range(4):
                        col = qT[:, ds(ac * BL + b, 1)]
                        if (b * 4 + ac) % 3 == 2:
                            nc.scalar.activation(
                                attT[:, ac, ds(b * LP, L)], fpT[:, ac, ds(b * LP, L)],
                                AF.Relu, bias=col)
                        else:
                            nc.vector.tensor_scalar(
                                attT[:, ac, ds(b * LP, L)], fpT[:, ac, ds(b * LP, L)],
                                col, 0.0, op0=OP.add, op1=OP.max)

                e_ps = [psMix.tile([128, L], f32, tag="mix", bufs=2, name=f"e_ps{i}") for i in range(2)]
                for b in range(BL):
                    h_, j = divmod(b, 4)
                    for kc in range(4):
                        nc.tensor.matmul(e_ps[h_][ds(32 * j, 1), :],
                                         wfull[:, kc:kc + 1],
                                         attT[:, kc, ds(b * LP, L)],
                                         start=(kc == 0), stop=(kc == 3),
                                         tile_position=(0, 32 * j))

                alT = [al0, al1]
                for h_ in range(2):
                    mx = work.tile([128, 1], f32, tag="mx")
                    nc.vector.tensor_reduce(mx[:, :], e_ps[h_][:, :], axis=X, op=OP.max)
                    nmx = work.tile([128, 1], f32, tag="nmx")
                    nc.vector.tensor_scalar_mul(nmx[:, :], mx[:, :], -1.0)
                    au = work.tile([128, L], f32, tag="au")
                    ssum = work.tile([128, 1], f32, tag="ssum")
                    nc.scalar.activation(au[:, :], e_ps[h_][:, :], AF.Exp,
                                         bias=nmx[:, :1], accum_out=ssum[:, :1])
                    rcp = work.tile([128, 1], f32, tag="rcp")
                    nc.vector.reciprocal(rcp[:, :], ssum[:, :])
                    nc.vector.tensor_scalar_mul(alT[h_][:, t, :], au[:, :], rcp[:, :1])

                for h_ in range(2):
                    tpl = psMix.tile([128, 128], bf16, tag="mixb", bufs=2)
                    nc.tensor.transpose(tpl[:, :], alT[h_][:, t, 0:128], idb[:, :])
                    tph = psMix.tile([128, 128], bf16, tag="mixb", bufs=2)
                    nc.tensor.transpose(tph[:68, :], alT[h_][:, t, 128:L], idb[:, :])
                    for j in range(4):
                        b = h_ * 4 + j
                        nc.vector.tensor_copy(bd[:, 2 * b, b:b + 1],
                                              tpl[:, ds(32 * j, 1)])
                        nc.vector.tensor_copy(bd[0:68, 2 * b + 1, b:b + 1],
                                              tph[0:68, ds(32 * j, 1)])

                z_ps = psMix.tile([BL, ENC], f32, tag="mix", bufs=2)
                for kc in range(16):
                    nc.tensor.matmul(z_ps[:, :], bd[:, kc, :], fstk[:, kc, :],
                                     start=(kc == 0), stop=(kc == 15))

                beta_ps = psMix.tile([BL, ENC], f32, tag="mix", bufs=2)
                for kc in range(4):
                    nc.tensor.matmul(beta_ps[:, :], hTs(kc), wbeta[:, kc, :],
                                     start=(kc == 0), stop=False)
                nc.tensor.matmul(beta_ps[:, :], onesb[:, :BL], bbeta[:, :],
                                 start=False, stop=True)
                tb = work.tile([BL, ENC], f32, tag="tb", bufs=1)
                nc.scalar.activation(tb[:, :], beta_ps[:, :], AF.Tanh, scale=0.5)
                sb = work.tile([BL, ENC], f32, tag="sb", bufs=1)
                nc.vector.tensor_scalar(sb[:, :], tb[:, :], 0.5, 0.5,
                                        op0=OP.mult, op1=OP.add)
                zg = work.tile([BL, ENC], bf16, tag="zg")
                nc.vector.tensor_tensor(zg[:, :], sb[:, :], z_ps[:, :], op=OP.mult)

                for ec in range(4):
                    tp = psMix.tile([128, 128], bf16, tag="mixb", bufs=2)
                    nc.tensor.transpose(tp[:, :BL], zg[:, ts(ec, 128)], idb[:BL, :BL])
                    nc.vector.tensor_copy(zT_all[:, ec, t, :], tp[:, :BL])

                gx = gxp.tile([BL, 4 * D], f32, tag="gx")
                nc.sync.dma_start(gx[:, :], gx_dram[ds(t * BL, BL), :])
                g_ps = psG.tile([BL, 4 * D], f32, tag="g", bufs=1)
                for nb in range(4):
                    for kc in range(4):
                        nc.tensor.matmul(g_ps[:, ts(nb, 512)], zT_all[:, kc, t, :],
                                         wzh[:, kc, ts(nb, 512)],
                                         start=(kc == 0), stop=False)
                    for kc in range(4):
                        nc.tensor.matmul(g_ps[:, ts(nb, 512)], hTs(kc),
                                         wzh[:, 4 + kc, ts(nb, 512)],
                                         start=False, stop=False)
                    nc.tensor.matmul(g_ps[:, ts(nb, 512)], idf[:BL, :BL],
                                     gx[:, ts(nb, 512)], start=False, stop=True)

                tg = work.tile([BL, 4 * D], f32, tag="tg", bufs=1)
                nc.scalar.activation(tg[:, 0:2 * D], g_ps[:, 0:2 * D], AF.Tanh, scale=0.5)
                nc.scalar.activation(tg[:, ds(2 * D, D)], g_ps[:, ds(2 * D, D)], AF.Tanh)
                nc.scalar.activation(tg[:, ds(3 * D, D)], g_ps[:, ds(3 * D, D)],
                                     AF.Tanh, scale=0.5)
                sg = work.tile([BL, 4 * D], f32, tag="sg", bufs=1)
                nc.vector.tensor_scalar(sg[:, 0:2 * D], tg[:, 0:2 * D], 0.5, 0.5,
                                        op0=OP.mult, op1=OP.add)
                nc.vector.tensor_scalar(sg[:, ds(3 * D, D)], tg[:, ds(3 * D, D)],
                                        0.5, 0.5, op0=OP.mult, op1=OP.add)
                t1 = work.tile([BL, D], f32, tag="t1")
                nc.vector.tensor_tensor(t1[:, :], sg[:, ds(D, D)], c_s[:, :], op=OP.mult)
                t2 = work.tile([BL, D], f32, tag="t2")
                nc.vector.tensor_tensor(t2[:, :], sg[:, 0:D], tg[:, ds(2 * D, D)],
                                        op=OP.mult)
                nc.vector.tensor_tensor(c_s[:, :], t1[:, :], t2[:, :], op=OP.add)
                tc_ = work.tile([BL, D], f32, tag="tc_")
                nc.scalar.activation(tc_[:, :], c_s[:, :], AF.Tanh)
                hb = work.tile([BL, D], bf16, tag="hb")
                nc.vector.tensor_tensor(hb[:, :], sg[:, ds(3 * D, D)], tc_[:, :],
                                        op=OP.mult)
                for ec in range(4):
                    tp = psMix.tile([128, 128], bf16, tag="mixb", bufs=2)
                    nc.tensor.transpose(tp[:, :BL], hb[:, ts(ec, 128)], idb[:BL, :BL])
                    nc.vector.tensor_copy(hT_all[:, ec, t, :], tp[:, :BL])

        # ---------------- end phase ----------------
        with tc.tile_pool(name="endp", bufs=2) as endp, \
             tc.tile_pool(name="edram", bufs=1, space="DRAM") as edram, \
             tc.tile_pool(name="psE", bufs=2, space="PSUM") as psE:
            ag_in = edram.tile([ROWS, E], f32)
            ag_out = edram.tile([NCORES * ROWS, E], f32, addr_space="Shared")
            for mc, nrow in [(0, 128), (1, 24)]:
                pp = psE.tile([128, E], f32, tag="pp", bufs=2)
                for kc in range(4):
                    nc.tensor.matmul(
                        pp[:nrow, :],
                        hT_all[:, kc, :, :].rearrange("p t b -> p (t b)")[:, ds(mc * 128, nrow)],
                        wlh[:, kc, :], start=(kc == 0), stop=False)
                for kc in range(4):
                    nc.tensor.matmul(
                        pp[:nrow, :],
                        zT_all[:, kc, :, :].rearrange("p t b -> p (t b)")[:, ds(mc * 128, nrow)],
                        wlz[:, kc, :], start=False, stop=False)
                nc.tensor.matmul(pp[:nrow, :], onesb[:, :nrow], blhz[:, :],
                                 start=False, stop=True)
                prj = endp.tile([128, E], f32, tag="prj")
                xt = x0 if mc == 0 else x1
                nc.vector.tensor_tensor(prj[:nrow, :], pp[:nrow, :], xt[:nrow, :],
                                        op=OP.add)
                nc.sync.dma_start(ag_in[ds(mc * 128, nrow), :], prj[:nrow, :])
                nc.sync.dma_start(io["dbg_proj"].ap()[ds(mc * 128, nrow), :], prj[:nrow, :])

            nc.gpsimd.collective_compute(
                "AllGather", mybir.AluOpType.bypass,
                replica_groups=[list(range(NCORES))],
                ins=[ag_in[:, :].opt()], outs=[ag_out[:, :].opt()])

            NR = NCORES * ROWS  # 1216
            projT = endp.tile([128, 4, NR], bf16, tag="projT", bufs=1)
            for mb in range(10):
                nrow = 128 if mb < 9 else NR - 9 * 128
                pa = endp.tile([128, E], f32, tag="pa")
                nc.sync.dma_start(pa[:nrow, :], ag_out[ds(mb * 128, nrow), :])
                nc.sync.dma_start(io["dbg_pag"].ap()[ds(mb * 128, nrow), :], pa[:nrow, :])
                for ec in range(4):
                    tp = psE.tile([128, 128], f32, tag="tp", bufs=2)
                    nc.tensor.transpose(tp[:, :nrow], pa[:nrow, ts(ec, 128)],
                                        idf[:nrow, :nrow])
                    nc.vector.tensor_copy(projT[:, ec, ds(mb * 128, nrow)],
                                          tp[:, :nrow])

            nsl = [(0, 512), (512, 512), (1024, VS - 1024)]
            for mb in range(10):
                nrow = 128 if mb < 9 else NR - 9 * 128
                for (n0, nw) in nsl:
                    lp = psE.tile([128, 512], f32, tag="lp", bufs=4)
                    for kc in range(4):
                        nc.tensor.matmul(lp[:nrow, :nw], projT[:, kc, ds(mb * 128, nrow)],
                                         wlo[:, kc, ds(n0, nw)],
                                         start=(kc == 0), stop=(kc == 3))
                    lo = endp.tile([128, 512], f32, tag="lo", bufs=3)
                    nc.vector.tensor_tensor(lo[:nrow, :nw], lp[:nrow, :nw],
                                            bloBC[:nrow, ds(n0, nw)], op=OP.add)
                    r0, r1 = mb * 128, mb * 128 + nrow
                    seg = r0
                    while seg < r1:
                        sc = seg // ROWS
                        send = min(r1, (sc + 1) * ROWS)
                        rem0 = seg - sc * ROWS
                        nseg = send - seg
                        tspan = nseg // BL
                        dst = io["logits_s"].ap()[ds(sc * BL, BL),
                                                  ds(rem0 // BL, tspan), ds(n0, nw)]
                        nc.sync.dma_start(
                            dst.rearrange("b t v -> t b v"),
                            lo[ds(seg - r0, nseg), :nw])
                        seg = send

            for h_, alt in [(0, al0), (1, al1)]:
                for j in range(4):
                    nc.gpsimd.dma_start(
                        io["alphas_s"].ap()[ds(h_ * 4 + j, 1), :, :],
                        alt[ds(32 * j, 1), :, :])


def _prep_inputs(inputs):
    f = {k: np.asarray(v) for k, v in inputs.items()}
    bfc = lambda x: np.ascontiguousarray(np.asarray(x, dtype=np.float32)).astype(_BF)
    f32c = lambda x: np.ascontiguousarray(np.asarray(x, dtype=np.float32))

    shared = {
        "emb": f32c(f["emb_table"]),
        "wenc": bfc(f["Wa_enc"]),
        "baed": f32c((f["ba_enc"] + f["ba_dec"]).reshape(4, 128).T),
        "wad": bfc(f["Wa_dec"]),
        "wfull": bfc(f["Wa_full"][:, 0].reshape(4, 128).T),
        "wbeta": bfc(f["W_beta"]),
        "bbeta": bfc(f["b_beta"].reshape(1, -1)),
        "wzh": bfc(np.concatenate([f["W_ih"][E:], f["W_hh"]], axis=0)),
        "wihx": bfc(f["W_ih"][:E]),
        "gbias": bfc((f["b_ih"] + f["b_hh"]).reshape(1, -1)),
        "winh": f32c(f["W_init_h"]),
        "binh": f32c(f["b_init_h"].reshape(1, -1)),
        "winc": f32c(f["W_init_c"]),
        "binc": f32c(f["b_init_c"].reshape(1, -1)),
        "wlh": bfc(f["W_Lh"]),
        "wlz": bfc(f["W_Lz"]),
        "blhz": bfc((f["b_Lh"] + f["b_Lz"]).reshape(1, -1)),
    }
    caps = np.asarray(f["captions"]).astype(np.int32)
    feats = f32c(f["features"])

    in_maps = []
    for c in range(NCORES):
        m = dict(shared)
        fs = np.zeros((BL * LP, ENC), np.float32)
        for b in range(BL):
            fs[b * LP: b * LP + L] = feats[c * BL + b]
        m["fstk"] = fs.astype(_BF)
        ci = np.zeros((ROWS, 1), np.int32)
        for t in range(NT):
            ci[t * BL:(t + 1) * BL, 0] = caps[c * BL:(c + 1) * BL, t]
        m["capi"] = ci
        m["wlo"] = bfc(f["W_Lo"][:, c * VS:(c + 1) * VS])
        m["blo"] = f32c(f["b_Lo"][c * VS:(c + 1) * VS].reshape(1, -1))
        in_maps.append(m)
    return in_maps


def kernel(**inputs):
    from concourse.bass_utils import run_bass_kernel_spmd
    if "nc" not in _CACHE:
        _CACHE["nc"] = _build()
    nc = _CACHE["nc"]
    in_maps = _prep_inputs(inputs)
    res = run_bass_kernel_spmd(nc, in_maps, core_ids=list(range(NCORES)))
    logits = np.concatenate([res.results[c]["logits_s"] for c in range(NCORES)], axis=2)
    alphas = np.concatenate([res.results[c]["alphas_s"] for c in range(NCORES)], axis=0)
    return logits, alphas


# revision 16
# speedup vs baseline: 1.1229x; 1.1229x over previous
"""Show-Attend-Tell decoder on 8 TRN2 NeuronCores.

Sharding: data-parallel recurrence (8 batches/core, attention+LSTM fully
local), deep-output/vocab projection batched at the END: one AllGather of
per-step proj inputs, then each core computes its V/8 = 1250 vocab shard
for all 64 batches with full-M matmuls.

Shapes (hardcoded): V=10000 E=D=A=ENC=512 B=64 L=196 T=20.
"""
import numpy as np
import ml_dtypes
from contextlib import ExitStack

V, E, D, A, ENC = 10000, 512, 512, 512, 512
B, L, T = 64, 196, 20
NCORES = 8
BL = B // NCORES          # 8 local batches
LP = 256                  # padded per-batch L
NT = T - 1                # 19 steps
ROWS = NT * BL            # 152 (t-major rows t*8+b)
VS = V // NCORES          # 1250 vocab shard

_BF = ml_dtypes.bfloat16
_CACHE = {}

# gate-column permutation: strip j holds [i_j|f_j|g_j|o_j]
# newcol j*512 + g*128 + d'  <-  oldcol g*512 + j*128 + d'
_GPERM = np.empty(4 * D, np.int64)
for _j in range(4):
    for _g in range(4):
        _GPERM[_j * 512 + _g * 128: _j * 512 + _g * 128 + 128] = \
            np.arange(_g * 512 + _j * 128, _g * 512 + _j * 128 + 128)


def _build(skip_collective=False):
    import concourse.mybir as mybir
    import concourse.tile as tile
    from concourse import bacc

    f32 = mybir.dt.float32
    bf16 = mybir.dt.bfloat16
    i32 = mybir.dt.int32

    nc = bacc.Bacc("TRN2", target_bir_lowering=False, debug=False,
                   num_devices=NCORES)

    io = {}
    def din(name, shape, dt):
        io[name] = nc.dram_tensor(name, shape, dt, kind="ExternalInput")
    din("fstk", [BL * LP, ENC], bf16)
    din("capi", [ROWS, 1], i32)
    din("emb", [V, E], f32)
    din("wenc", [ENC, A], bf16)
    din("baed", [128, 4], f32)
    din("wad", [D, A], bf16)
    din("wfull", [128, 4], bf16)
    din("wbeta", [D, ENC], bf16)
    din("bbeta", [1, ENC], bf16)
    din("wzh", [ENC + D, 4 * D], bf16)
    din("wihx", [E, 4 * D], bf16)
    din("gbias", [1, 4 * D], bf16)
    din("winh", [ENC, D], bf16)
    din("binh", [1, D], bf16)
    din("winc", [ENC, D], bf16)
    din("binc", [1, D], bf16)
    din("wlh", [D, E], bf16)
    din("wlz", [ENC, E], bf16)
    din("blhz", [1, E], bf16)
    din("wlo", [E, VS], bf16)
    din("blo", [1, VS], f32)
    io["logits_s"] = nc.dram_tensor("logits_s", [B, NT, VS], f32, kind="ExternalOutput")
    io["alphas_s"] = nc.dram_tensor("alphas_s", [BL, NT, L], f32, kind="ExternalOutput")

    with tile.TileContext(nc) as tc:
        _body(nc, tc, io, skip_collective)
    nc.compile()
    return nc


def _body(nc, tc, io, skip_collective=False):
    import concourse.bass as bass
    import concourse.mybir as mybir

    f32 = mybir.dt.float32
    bf16 = mybir.dt.bfloat16
    i32 = mybir.dt.int32
    AF = mybir.ActivationFunctionType
    OP = mybir.AluOpType
    ds, ts = bass.ds, bass.ts
    X = mybir.AxisListType.X

    with ExitStack() as stk:
        const = stk.enter_context(tc.tile_pool(name="const", bufs=1))
        wpool = stk.enter_context(tc.tile_pool(name="wpool", bufs=1))
        state = stk.enter_context(tc.tile_pool(name="state", bufs=1))
        dram = stk.enter_context(tc.tile_pool(name="dram", bufs=1, space="DRAM"))

        from concourse.masks import make_identity
        idf = const.tile([128, 128], f32)
        make_identity(nc, idf)
        idb = const.tile([128, 128], bf16)
        make_identity(nc, idb)
        onesb = const.tile([1, 128], bf16)
        nc.gpsimd.memset(onesb[:, :], 1.0)
        onesf = const.tile([1, 8], f32)
        nc.gpsimd.memset(onesf[:, :], 1.0)

        def dma_cpe(dst, name):
            nc.sync.dma_start(dst[:, :, :], io[name].ap().rearrange("(c p) e -> p c e", p=128))

        fstk = wpool.tile([128, 16, ENC], bf16); dma_cpe(fstk, "fstk")
        wad = wpool.tile([128, 4, A], bf16); dma_cpe(wad, "wad")
        wbeta = wpool.tile([128, 4, ENC], bf16); dma_cpe(wbeta, "wbeta")
        wzh = wpool.tile([128, 8, 4 * D], bf16); dma_cpe(wzh, "wzh")
        wlh = wpool.tile([128, 4, E], bf16); dma_cpe(wlh, "wlh")
        wlz = wpool.tile([128, 4, E], bf16); dma_cpe(wlz, "wlz")
        wlo = wpool.tile([128, 4, VS], bf16); dma_cpe(wlo, "wlo")
        wfull = wpool.tile([128, 4], bf16)
        nc.sync.dma_start(wfull[:, :], io["wfull"].ap())
        baed = wpool.tile([128, 4], f32)
        nc.sync.dma_start(baed[:, :], io["baed"].ap())
        bbeta = wpool.tile([1, ENC], bf16)
        nc.sync.dma_start(bbeta[:, :], io["bbeta"].ap())
        blhz = wpool.tile([1, E], bf16)
        nc.sync.dma_start(blhz[:, :], io["blhz"].ap())
        bloBC = wpool.tile([128, VS], f32)
        blo_row = wpool.tile([1, VS], f32)
        nc.sync.dma_start(blo_row[:, :], io["blo"].ap())
        nc.gpsimd.partition_broadcast(bloBC[:, :], blo_row[:, :])

        fpT = state.tile([128, 4, BL * LP], bf16)
        x0 = state.tile([128, ENC], f32)
        x1 = state.tile([24, ENC], f32)
        xT = state.tile([128, 4, ROWS], bf16)
        hT_all = state.tile([128, NT, 128], bf16)   # cols 32*kc + b
        zT_all = state.tile([128, NT, 128], bf16)
        hT0 = state.tile([128, 128], bf16)
        c_s = state.tile([128, 128], f32)           # rows 32j+b, cols d-within-strip
        al0 = state.tile([128, NT, L], bf16)
        al1 = state.tile([128, NT, L], bf16)
        bd = state.tile([128, 16, BL], bf16)
        nc.gpsimd.memset(bd[:, :, :], 0.0)

        gx_dram = dram.tile([ROWS, 4 * D], bf16)

        # ---------------- preamble ----------------
        with tc.tile_pool(name="pre", bufs=1) as pre, \
             tc.tile_pool(name="prep", bufs=2, space="PSUM") as prep:
            idx0 = pre.tile([128, 1], i32)
            idx1 = pre.tile([24, 1], i32)
            nc.sync.dma_start(idx0[:, :], io["capi"].ap()[0:128, :])
            nc.sync.dma_start(idx1[:, :], io["capi"].ap()[128:ROWS, :])
            nc.gpsimd.indirect_dma_start(
                out=x0[:, :], out_offset=None, in_=io["emb"].ap(),
                in_offset=bass.IndirectOffsetOnAxis(ap=idx0[:, :1], axis=0))
            nc.gpsimd.indirect_dma_start(
                out=x1[:, :], out_offset=None, in_=io["emb"].ap(),
                in_offset=bass.IndirectOffsetOnAxis(ap=idx1[:, :1], axis=0))
            for rc, (xt, nrow) in enumerate([(x0, 128), (x1, 24)]):
                for ec in range(4):
                    tp = prep.tile([128, 128], f32, tag="tp", bufs=2)
                    nc.tensor.transpose(tp[:, :nrow], xt[:nrow, ts(ec, 128)],
                                        idf[:nrow, :nrow])
                    nc.vector.tensor_copy(xT[:, ec, ds(rc * 128, nrow)], tp[:, :nrow])

            featT = pre.tile([128, 4, BL * LP], bf16)
            for c in range(16):
                for ec in range(4):
                    tp = prep.tile([128, 128], bf16, tag="tpb", bufs=2)
                    nc.tensor.transpose(tp[:, :], fstk[:, c, ts(ec, 128)], idb[:, :])
                    nc.vector.tensor_copy(featT[:, ec, ts(c, 128)], tp[:, :])

            wenc = pre.tile([128, 4, A], bf16)
            nc.sync.dma_start(wenc[:, :, :], io["wenc"].ap().rearrange("(c p) e -> p c e", p=128))
            for ac in range(4):
                for nb in range(4):
                    pp = prep.tile([128, 512], f32, tag="pp", bufs=2)
                    for kc in range(4):
                        nc.tensor.matmul(pp[:, :], wenc[:, kc, ts(ac, 128)],
                                         featT[:, kc, ts(nb, 512)],
                                         start=(kc == 0), stop=(kc == 3))
                    if (ac + nb) % 2:
                        nc.scalar.activation(fpT[:, ac, ts(nb, 512)], pp[:, :],
                                             AF.Identity, bias=baed[:, ac:ac + 1])
                    else:
                        nc.vector.tensor_scalar(fpT[:, ac, ts(nb, 512)], pp[:, :],
                                                baed[:, ac:ac + 1], None, op0=OP.add)

            wihx = pre.tile([128, 4, 4 * D], bf16)
            nc.sync.dma_start(wihx[:, :, :], io["wihx"].ap().rearrange("(c p) e -> p c e", p=128))
            gbias = pre.tile([1, 4 * D], bf16)
            nc.sync.dma_start(gbias[:, :], io["gbias"].ap())
            for mc, nrow in [(0, 128), (1, 24)]:
                stg = pre.tile([128, 4 * D], bf16, tag="gstage")
                for nb in range(4):
                    pp = prep.tile([128, 512], f32, tag="pp", bufs=2)
                    for kc in range(4):
                        nc.tensor.matmul(pp[:nrow, :], xT[:, kc, ds(mc * 128, nrow)],
                                         wihx[:, kc, ts(nb, 512)],
                                         start=(kc == 0), stop=False)
                    nc.tensor.matmul(pp[:nrow, :], onesb[:, :nrow],
                                     gbias[:, ts(nb, 512)], start=False, stop=True)
                    nc.vector.tensor_copy(stg[:nrow, ts(nb, 512)], pp[:nrow, :])
                nc.sync.dma_start(gx_dram[ds(mc * 128, nrow), :], stg[:nrow, :])

            bdo = pre.tile([128, 16, BL], bf16)
            nc.gpsimd.memset(bdo[:, :, :], 0.0)
            for b in range(BL):
                nc.gpsimd.memset(bdo[:, 2 * b, b:b + 1], 1.0 / L)
                nc.gpsimd.memset(bdo[0:L - 128, 2 * b + 1, b:b + 1], 1.0 / L)
            mean_ps = prep.tile([BL, ENC], f32, tag="pz", bufs=2)
            for kc in range(16):
                nc.tensor.matmul(mean_ps[:, :], bdo[:, kc, :], fstk[:, kc, :],
                                 start=(kc == 0), stop=(kc == 15))
            mean_s = pre.tile([BL, ENC], f32)
            nc.vector.tensor_copy(mean_s[:, :], mean_ps[:, :])
            meanT = pre.tile([128, 4, BL], bf16)
            for ec in range(4):
                tp = prep.tile([128, 128], f32, tag="tp", bufs=2)
                nc.tensor.transpose(tp[:, :BL], mean_s[:, ts(ec, 128)], idf[:BL, :BL])
                nc.vector.tensor_copy(meanT[:, ec, :], tp[:, :BL])
            winh = pre.tile([128, 4, D], bf16)
            nc.sync.dma_start(winh[:, :, :], io["winh"].ap().rearrange("(c p) e -> p c e", p=128))
            winc = pre.tile([128, 4, D], bf16)
            nc.sync.dma_start(winc[:, :, :], io["winc"].ap().rearrange("(c p) e -> p c e", p=128))
            binh = pre.tile([1, D], bf16)
            nc.sync.dma_start(binh[:, :], io["binh"].ap())
            binc = pre.tile([1, D], bf16)
            nc.sync.dma_start(binc[:, :], io["binc"].ap())
            h0b = pre.tile([128, 128], bf16)
            for w_, b_, is_h in [(winh, binh, True), (winc, binc, False)]:
                pp = prep.tile([128, 128], f32, tag="pz", bufs=2)
                for j in range(4):
                    for kc in range(4):
                        nc.tensor.matmul(pp[ds(32 * j, BL), :], meanT[:, kc, :],
                                         w_[:, kc, ts(j, 128)],
                                         start=(kc == 0), stop=False,
                                         tile_position=(0, 32 * j))
                    nc.tensor.matmul(pp[ds(32 * j, BL), :], onesb[:, :BL],
                                     b_[:, ts(j, 128)], start=False, stop=True,
                                     tile_position=(0, 32 * j))
                if is_h:
                    nc.scalar.activation(h0b[:, :], pp[:, :], AF.Tanh)
                else:
                    nc.scalar.activation(c_s[:, :], pp[:, :], AF.Tanh)
            tp = prep.tile([128, 128], bf16, tag="tpb", bufs=2)
            nc.tensor.transpose(tp[:, :], h0b[:, :], idb[:, :])
            nc.vector.tensor_copy(hT0[:, :], tp[:, :])

            dummy = pre.tile([1, 1], f32)
            nc.scalar.activation(dummy[:, :], dummy[:, :], AF.Exp)

        # ---------------- recurrence ----------------
        with tc.tile_pool(name="work", bufs=2) as work, \
             tc.tile_pool(name="gxp", bufs=2) as gxp, \
             tc.tile_pool(name="psMix", bufs=2, space="PSUM") as psMix, \
             tc.tile_pool(name="psG", bufs=2, space="PSUM") as psG:
            for t in range(NT):
                hT = hT0 if t == 0 else hT_all[:, t - 1, :]

                # qT[a, b] = Wa_dec^T @ h  (stationary Wa_dec tiles)
                qT_ps = psMix.tile([128, 4 * BL], f32, tag="mix", bufs=2)
                for ac in range(4):
                    for kc in range(4):
                        nc.tensor.matmul(qT_ps[:, ds(ac * BL, BL)],
                                         wad[:, kc, ts(ac, 128)],
                                         hT[:, ds(32 * kc, BL)],
                                         start=(kc == 0), stop=(kc == 3))
                qT = work.tile([128, 4 * BL], f32, tag="qT")
                nc.vector.tensor_copy(qT[:, :], qT_ps[:, :])

                # attT = relu(fpT + qT[:,b])  (32 ops, DVE/ACT/GPSIMD split)
                attT = work.tile([128, 4, BL * LP], bf16, tag="attT", bufs=1)
                for b in range(BL):
                    for ac in range(4):
                        col = qT[:, ds(ac * BL + b, 1)]
                        k = (b * 4 + ac) % 3
                        if k == 0:
                            nc.scalar.activation(
                                attT[:, ac, ds(b * LP, L)], fpT[:, ac, ds(b * LP, L)],
                                AF.Relu, bias=col)
                        elif k == 1:
                            nc.vector.tensor_scalar(
                                attT[:, ac, ds(b * LP, L)], fpT[:, ac, ds(b * LP, L)],
                                col, 0.0, op0=OP.add, op1=OP.max)
                        else:
                            nc.gpsimd.tensor_scalar(
                                attT[:, ac, ds(b * LP, L)], fpT[:, ac, ds(b * LP, L)],
                                col, 0.0, op0=OP.add, op1=OP.max)

                # e[b, l] (per-b reduce over A; col-strip packed by psum row 32j)
                e_ps = [psMix.tile([128, L], f32, tag="mix", bufs=2, name=f"e_ps{i}") for i in range(2)]
                for b in range(BL):
                    h_, j = divmod(b, 4)
                    for kc in range(4):
                        nc.tensor.matmul(e_ps[h_][ds(32 * j, 1), :],
                                         wfull[:, kc:kc + 1],
                                         attT[:, kc, ds(b * LP, L)],
                                         start=(kc == 0), stop=(kc == 3),
                                         tile_position=(0, 32 * j))

                alT = [al0, al1]
                for h_ in range(2):
                    mx = work.tile([128, 1], f32, tag="mx")
                    nc.vector.tensor_reduce(mx[:, :], e_ps[h_][:, :], axis=X, op=OP.max)
                    nmx = work.tile([128, 1], f32, tag="nmx")
                    nc.vector.tensor_scalar_mul(nmx[:, :], mx[:, :], -1.0)
                    au = work.tile([128, L], f32, tag="au")
                    ssum = work.tile([128, 1], f32, tag="ssum")
                    nc.scalar.activation(au[:, :], e_ps[h_][:, :], AF.Exp,
                                         bias=nmx[:, :1], accum_out=ssum[:, :1])
                    rcp = work.tile([128, 1], f32, tag="rcp")
                    nc.vector.reciprocal(rcp[:, :], ssum[:, :])
                    nc.vector.tensor_scalar_mul(alT[h_][:, t, :], au[:, :], rcp[:, :1])

                # alphaT -> bd columns (strided: flat col 17*b (+8 for hi))
                bdf = bd[:, :, :].rearrange("p c e -> p (c e)")
                for h_ in range(2):
                    tpl = psMix.tile([128, 128], bf16, tag="mixb", bufs=2)
                    nc.tensor.transpose(tpl[:, :], alT[h_][:, t, 0:128], idb[:, :])
                    tph = psMix.tile([128, 128], bf16, tag="mixb", bufs=2)
                    nc.tensor.transpose(tph[:68, :], alT[h_][:, t, 128:L], idb[:, :])
                    o_ = h_ * 4 * 17
                    nc.vector.tensor_copy(bdf[:, o_:o_ + 52:17], tpl[:, 0:128:32])
                    nc.vector.tensor_copy(bdf[0:68, o_ + 8:o_ + 60:17], tph[0:68, 0:128:32])

                # z = bd^T @ fstack  [8, 512]
                z_ps = psMix.tile([BL, ENC], f32, tag="mix", bufs=2)
                for kc in range(16):
                    nc.tensor.matmul(z_ps[:, :], bd[:, kc, :], fstk[:, kc, :],
                                     start=(kc == 0), stop=(kc == 15))

                # beta = sigmoid(h @ W_beta + b_beta) via tanh
                beta_ps = psMix.tile([BL, ENC], f32, tag="mix", bufs=2)
                for kc in range(4):
                    nc.tensor.matmul(beta_ps[:, :], hT[:, ds(32 * kc, BL)],
                                     wbeta[:, kc, :], start=(kc == 0), stop=False)
                nc.tensor.matmul(beta_ps[:, :], onesb[:, :BL], bbeta[:, :],
                                 start=False, stop=True)
                tb = work.tile([BL, ENC], f32, tag="tb", bufs=1)
                nc.scalar.activation(tb[:, :], beta_ps[:, :], AF.Tanh, scale=0.5)
                sb = work.tile([BL, ENC], f32, tag="sb", bufs=1)
                nc.vector.tensor_scalar(sb[:, :], tb[:, :], 0.5, 0.5,
                                        op0=OP.mult, op1=OP.add)
                zg = work.tile([BL, ENC], bf16, tag="zg")
                nc.vector.tensor_tensor(zg[:, :], sb[:, :], z_ps[:, :], op=OP.mult)

                # zT -> zT_all[:, t, 32*ec + b]
                for ec in range(4):
                    tp = psMix.tile([128, 128], bf16, tag="mixb", bufs=2)
                    nc.tensor.transpose(tp[:, :BL], zg[:, ts(ec, 128)], idb[:BL, :BL])
                    nc.vector.tensor_copy(zT_all[:, t, ds(32 * ec, BL)], tp[:, :BL])

                # gates (strip-packed): strip j computes [i_j|f_j|g_j|o_j] at rows 32j
                gx = gxp.tile([BL, 4 * D], bf16, tag="gx")
                nc.sync.dma_start(gx[:, :], gx_dram[ds(t * BL, BL), :])
                g_ps = psG.tile([128, 512], f32, tag="g", bufs=2)
                for j in range(4):
                    for kc in range(4):
                        nc.tensor.matmul(g_ps[ds(32 * j, BL), :],
                                         zT_all[:, t, ds(32 * kc, BL)],
                                         wzh[:, kc, ts(j, 512)],
                                         start=(kc == 0), stop=False,
                                         tile_position=(0, 32 * j))
                    for kc in range(4):
                        nc.tensor.matmul(g_ps[ds(32 * j, BL), :],
                                         hT[:, ds(32 * kc, BL)],
                                         wzh[:, 4 + kc, ts(j, 512)],
                                         start=False, stop=False,
                                         tile_position=(0, 32 * j))
                    nc.tensor.matmul(g_ps[ds(32 * j, BL), :], idb[:BL, :BL],
                                     gx[:, ts(j, 512)], start=False, stop=True,
                                     tile_position=(0, 32 * j))

                # LSTM elementwise in strip layout (i:0-127 f:128-255 g:256-383 o:384-511)
                tg = work.tile([128, 512], f32, tag="tg", bufs=1)
                nc.scalar.activation(tg[:, 0:256], g_ps[:, 0:256], AF.Tanh, scale=0.5)
                nc.scalar.activation(tg[:, 256:384], g_ps[:, 256:384], AF.Tanh)
                nc.scalar.activation(tg[:, 384:512], g_ps[:, 384:512], AF.Tanh, scale=0.5)
                sg = work.tile([128, 512], f32, tag="sg", bufs=1)
                nc.vector.tensor_scalar(sg[:, 0:256], tg[:, 0:256], 0.5, 0.5,
                                        op0=OP.mult, op1=OP.add)
                nc.vector.tensor_scalar(sg[:, 384:512], tg[:, 384:512], 0.5, 0.5,
                                        op0=OP.mult, op1=OP.add)
                t1 = work.tile([128, 128], f32, tag="t1")
                nc.vector.tensor_tensor(t1[:, :], sg[:, 128:256], c_s[:, :], op=OP.mult)
                t2 = work.tile([128, 128], f32, tag="t2")
                nc.vector.tensor_tensor(t2[:, :], sg[:, 0:128], tg[:, 256:384],
                                        op=OP.mult)
                nc.vector.tensor_tensor(c_s[:, :], t1[:, :], t2[:, :], op=OP.add)
                tc_ = work.tile([128, 128], f32, tag="tc_")
                nc.scalar.activation(tc_[:, :], c_s[:, :], AF.Tanh)
                hb = work.tile([128, 128], bf16, tag="hb")
                nc.vector.tensor_tensor(hb[:, :], sg[:, 384:512], tc_[:, :],
                                        op=OP.mult)
                tp = psMix.tile([128, 128], bf16, tag="mixb", bufs=2)
                nc.tensor.transpose(tp[:, :], hb[:, :], idb[:, :])
                nc.vector.tensor_copy(hT_all[:, t, :], tp[:, :])

        # ---------------- end phase ----------------
        with tc.tile_pool(name="endp", bufs=2) as endp, \
             tc.tile_pool(name="edram", bufs=1, space="DRAM") as edram, \
             tc.tile_pool(name="psE", bufs=2, space="PSUM") as psE:
            ag_in = edram.tile([ROWS, E], bf16)
            ag_out = edram.tile([NCORES * ROWS, E], bf16, addr_space="Shared")
            for mc, nrow in [(0, 128), (1, 24)]:
                pp = psE.tile([128, E], f32, tag="pp", bufs=2)
                t0_, t1_ = (0, 16) if mc == 0 else (16, NT)
                hs = endp.tile([128, 4, 128], bf16, tag="hs", bufs=1)
                zs = endp.tile([128, 4, 128], bf16, tag="zs", bufs=1)
                for kc in range(4):
                    nc.vector.tensor_copy(hs[:, kc, :nrow],
                                          hT_all[:, t0_:t1_, ds(32 * kc, BL)])
                    nc.vector.tensor_copy(zs[:, kc, :nrow],
                                          zT_all[:, t0_:t1_, ds(32 * kc, BL)])
                for kc in range(4):
                    nc.tensor.matmul(pp[:nrow, :], hs[:, kc, :nrow],
                                     wlh[:, kc, :], start=(kc == 0), stop=False)
                for kc in range(4):
                    nc.tensor.matmul(pp[:nrow, :], zs[:, kc, :nrow],
                                     wlz[:, kc, :], start=False, stop=False)
                nc.tensor.matmul(pp[:nrow, :], onesb[:, :nrow], blhz[:, :],
                                 start=False, stop=True)
                prj = endp.tile([128, E], bf16, tag="prj")
                xt = x0 if mc == 0 else x1
                nc.vector.tensor_tensor(prj[:nrow, :], pp[:nrow, :], xt[:nrow, :],
                                        op=OP.add)
                nc.sync.dma_start(ag_in[ds(mc * 128, nrow), :], prj[:nrow, :])

            if not skip_collective:
                nc.gpsimd.collective_compute(
                    "AllGather", mybir.AluOpType.bypass,
                    replica_groups=[list(range(NCORES))],
                    ins=[ag_in[:, :].opt()], outs=[ag_out[:, :].opt()])

            NR = NCORES * ROWS  # 1216
            projT = endp.tile([128, 4, NR], bf16, tag="projT", bufs=1)
            for mb in range(10):
                nrow = 128 if mb < 9 else NR - 9 * 128
                pa = endp.tile([128, E], bf16, tag="pa")
                nc.sync.dma_start(pa[:nrow, :], ag_out[ds(mb * 128, nrow), :])
                for ec in range(4):
                    tp = psE.tile([128, 128], bf16, tag="tp", bufs=2)
                    nc.tensor.transpose(tp[:, :nrow], pa[:nrow, ts(ec, 128)],
                                        idb[:nrow, :nrow])
                    nc.vector.tensor_copy(projT[:, ec, ds(mb * 128, nrow)],
                                          tp[:, :nrow])

            nsl = [(0, 512), (512, 512), (1024, VS - 1024)]
            for mb in range(10):
                nrow = 128 if mb < 9 else NR - 9 * 128
                for (n0, nw) in nsl:
                    lp = psE.tile([128, 512], f32, tag="lp", bufs=4)
                    for kc in range(4):
                        nc.tensor.matmul(lp[:nrow, :nw], projT[:, kc, ds(mb * 128, nrow)],
                                         wlo[:, kc, ds(n0, nw)],
                                         start=(kc == 0), stop=(kc == 3))
                    lo = endp.tile([128, 512], f32, tag="lo", bufs=3)
                    nc.vector.tensor_tensor(lo[:nrow, :nw], lp[:nrow, :nw],
                                            bloBC[:nrow, ds(n0, nw)], op=OP.add)
                    r0, r1 = mb * 128, mb * 128 + nrow
                    seg = r0
                    while seg < r1:
                        sc = seg // ROWS
                        send = min(r1, (sc + 1) * ROWS)
                        rem0 = seg - sc * ROWS
                        nseg = send - seg
                        tspan = nseg // BL
                        dst = io["logits_s"].ap()[ds(sc * BL, BL),
                                                  ds(rem0 // BL, tspan), ds(n0, nw)]
                        nc.sync.dma_start(
                            dst.rearrange("b t v -> t b v"),
                            lo[ds(seg - r0, nseg), :nw])
                        seg = send

            for h_, alt in [(0, al0), (1, al1)]:
                for j in range(4):
                    nc.gpsimd.dma_start(
                        io["alphas_s"].ap()[ds(h_ * 4 + j, 1), :, :],
                        alt[ds(32 * j, 1), :, :])


def _prep_inputs(inputs):
    f = {k: np.asarray(v) for k, v in inputs.items()}
    bfc = lambda x: np.ascontiguousarray(np.asarray(x, dtype=np.float32)).astype(_BF)
    f32c = lambda x: np.ascontiguousarray(np.asarray(x, dtype=np.float32))

    shared = {
        "emb": f32c(f["emb_table"]),
        "wenc": bfc(f["Wa_enc"]),
        "baed": f32c((f["ba_enc"] + f["ba_dec"]).reshape(4, 128).T),
        "wad": bfc(f["Wa_dec"]),
        "wfull": bfc(f["Wa_full"][:, 0].reshape(4, 128).T),
        "wbeta": bfc(f["W_beta"]),
        "bbeta": bfc(f["b_beta"].reshape(1, -1)),
        "wzh": bfc(np.concatenate([f["W_ih"][E:], f["W_hh"]], axis=0)[:, _GPERM]),
        "wihx": bfc(f["W_ih"][:E][:, _GPERM]),
        "gbias": bfc((f["b_ih"] + f["b_hh"])[_GPERM].reshape(1, -1)),
        "winh": bfc(f["W_init_h"]),
        "binh": bfc(f["b_init_h"].reshape(1, -1)),
        "winc": bfc(f["W_init_c"]),
        "binc": bfc(f["b_init_c"].reshape(1, -1)),
        "wlh": bfc(f["W_Lh"]),
        "wlz": bfc(f["W_Lz"]),
        "blhz": bfc((f["b_Lh"] + f["b_Lz"]).reshape(1, -1)),
    }
    caps = np.asarray(f["captions"]).astype(np.int32)
    feats = f32c(f["features"])

    in_maps = []
    for c in range(NCORES):
        m = dict(shared)
        fs = np.zeros((BL * LP, ENC), np.float32)
        for b in range(BL):
            fs[b * LP: b * LP + L] = feats[c * BL + b]
        m["fstk"] = fs.astype(_BF)
        ci = np.zeros((ROWS, 1), np.int32)
        for t in range(NT):
            ci[t * BL:(t + 1) * BL, 0] = caps[c * BL:(c + 1) * BL, t]
        m["capi"] = ci
        m["wlo"] = bfc(f["W_Lo"][:, c * VS:(c + 1) * VS])
        m["blo"] = f32c(f["b_Lo"][c * VS:(c + 1) * VS].reshape(1, -1))
        in_maps.append(m)
    return in_maps


def kernel(**inputs):
    from concourse.bass_utils import run_bass_kernel_spmd
    if "nc" not in _CACHE:
        _CACHE["nc"] = _build()
    nc = _CACHE["nc"]
    in_maps = _prep_inputs(inputs)
    res = run_bass_kernel_spmd(nc, in_maps, core_ids=list(range(NCORES)))
    logits = np.concatenate([res.results[c]["logits_s"] for c in range(NCORES)], axis=2)
    alphas = np.concatenate([res.results[c]["alphas_s"] for c in range(NCORES)], axis=0)
    return logits, alphas
